# revision 1
# baseline (speedup 1.0000x reference)
"""Expert-parallel CMoE kernel for 8 Trainium2 NeuronCores.

Strategy (hardcoded for B=8, T=2048, D=1024, F=2048, E=16, C=1024):
  - core k owns batch k (data-parallel token shift + receptance + output)
    and experts {2k, 2k+1} (expert-parallel FFN).
  - hash routing is pure int math on token_ids -> computed on host; the
    resulting permutations are shipped to the cores as index tensors.
  - dispatch/combine are AllToAll collectives over a [8, K, D] bf16 buffer
    (K = max tokens any (src, dst) core pair exchanges, host-computed).
  - on-device dataflow per core:
      token shift (fp32 DVE) -> scatter packed xk rows into A2A#1 input
      AllToAll #1 (dispatch)
      receptance r = sigmoid(xr @ w_recept.T) overlaps the collective
      per expert: transposing-gather -> FFN1 -> relu^2 -> FFN2 ->
                  scatter y rows into A2A#2 input
      AllToAll #2 (combine)
      gather own tokens' y rows, multiply by r, write fp32 output.
  - matmuls/activations in bf16 with fp32 PSUM accumulation.
"""
import sys

for _p in ("/opt/trn_rl_repo", "/root/.axon_site/_ro/trn_rl_repo"):
    if _p not in sys.path:
        sys.path.append(_p)

import numpy as np
import ml_dtypes

import concourse.bass as bass
import concourse.bacc as bacc
import concourse.mybir as mybir
import concourse.tile as tile
from concourse.bass_utils import run_bass_kernel_spmd

P = 128
B, T, D, F, E = 8, 2048, 1024, 2048, 16
N = B * T
C = max(4, N // E)          # 1024
HASH_PRIME = 5099
NCORES = 8
EPC = E // NCORES           # experts per core = 2
BF16 = mybir.dt.bfloat16
F32 = mybir.dt.float32
I16 = mybir.dt.int16
I32 = mybir.dt.int32
nbf16 = ml_dtypes.bfloat16
AF = mybir.ActivationFunctionType

_CACHE = {}


# ----------------------------------------------------------------- host routing
def _route(token_ids):
    tid = np.asarray(token_ids).reshape(N).astype(np.int64)
    e = (tid * HASH_PRIME) % E
    onehot = (e[:, None] == np.arange(E)).astype(np.int64)
    pos = onehot.cumsum(0)[np.arange(N), e] - 1
    keep = pos < C
    return e, pos, keep


def _build_indices(token_ids):
    """Returns (K, per-core index tensors)."""
    e, pos, keep = _route(token_ids)
    src = np.arange(N) // T              # owning core of each token
    dst = e // EPC                       # expert core of each token

    # rank of each kept token within its (src, dst) pair, ordered by (e, pos)
    order = np.lexsort((pos, e, dst, src))
    rank = np.zeros(N, np.int64)
    cnt = np.zeros((NCORES, NCORES), np.int64)
    for n in order:
        if keep[n]:
            rank[n] = cnt[src[n], dst[n]]
            cnt[src[n], dst[n]] += 1
    K = int(-(-cnt.max() // 16) * 16)    # round up to 16
    RT = NCORES * K                      # trash/zero row index

    # src_idx[n]: packed position of token n inside its core's A2A#1 input
    src_idx = np.where(keep, dst * K + rank, RT)
    # recv_idx[(dst core), el*C + c]: row in recv1 holding slot (el, c)'s token
    # a2a2 scatter uses the same values; empty slots -> RT (zero row / trash)
    recv_idx = np.full((NCORES, EPC * C), RT, np.int64)
    for n in range(N):
        if keep[n]:
            el = e[n] - dst[n] * EPC
            recv_idx[dst[n], el * C + pos[n]] = src[n] * K + rank[n]

    def wrap16(a):
        a = np.asarray(a, np.int16)
        w = a.reshape(-1, 16).T.copy()       # [16, n/16], j at [j%16, j//16]
        return np.tile(w, (8, 1))            # replicate across 8 Q7 cores

    per_core = []
    for k in range(NCORES):
        tok = slice(k * T, (k + 1) * T)
        per_core.append({
            "src_idx32": src_idx[tok].astype(np.int32).reshape(T // P, P).T.copy(),
            "src_idx16": wrap16(src_idx[tok]),
            "slot_idx16": wrap16(recv_idx[k]),
            "slot_idx32": recv_idx[k].astype(np.int32).reshape(-1, P).T.copy(),
        })
    return K, per_core


# ----------------------------------------------------------------- device kernel
def _build_nc(K):
    RT = NCORES * K
    nc = bacc.Bacc("TRN2", target_bir_lowering=False, debug=False,
                   num_devices=NCORES)

    x_ext = nc.dram_tensor("x_ext", [T + 1, D], F32, kind="ExternalInput")
    maa_k = nc.dram_tensor("maa_k", [1, D], F32, kind="ExternalInput")
    maa_r = nc.dram_tensor("maa_r", [1, D], F32, kind="ExternalInput")
    wrt = nc.dram_tensor("wrt", [D, D], BF16, kind="ExternalInput")
    wk = nc.dram_tensor("wk", [EPC, D, F], BF16, kind="ExternalInput")
    wv = nc.dram_tensor("wv", [EPC, F, D], BF16, kind="ExternalInput")
    src_idx32 = nc.dram_tensor("src_idx32", [P, T // P], I32, kind="ExternalInput")
    src_idx16 = nc.dram_tensor("src_idx16", [P, T // 16], I16, kind="ExternalInput")
    slot_idx16 = nc.dram_tensor("slot_idx16", [P, EPC * C // 16], I16,
                                kind="ExternalInput")
    slot_idx32 = nc.dram_tensor("slot_idx32", [P, EPC * C // P], I32,
                                kind="ExternalInput")
    iota16 = nc.dram_tensor("iota16", [P, T // 16], I16, kind="ExternalInput")
    out = nc.dram_tensor("out", [T, D], F32, kind="ExternalOutput")

    DC = D // P          # 8 d-chunks
    FC = F // P          # 16 f-chunks
    rg = [list(range(NCORES))]

    with tile.TileContext(nc) as tc:
        with (
            tc.tile_pool(name="dram", bufs=1, space="DRAM") as dram,
            tc.tile_pool(name="misc", bufs=1) as misc,
            tc.tile_pool(name="psr", bufs=2, space="PSUM") as psr,
            tc.tile_pool(name="psh", bufs=2, space="PSUM") as psh,
            tc.tile_pool(name="psy", bufs=2, space="PSUM") as psy,
        ):
            a2a1_in = dram.tile([RT + 1, D], BF16)
            recv1 = dram.tile([RT + 1, D], BF16)
            a2a2_in = dram.tile([RT + 1, D], BF16)
            recv2 = dram.tile([RT + 1, D], BF16)
            xr_buf = dram.tile([T, D], BF16)
            r_buf = dram.tile([T, D], BF16)

            # zero rows for dropped-token / empty-slot reads
            zrow = misc.tile([1, D], BF16)
            nc.vector.memzero(zrow[:])
            nc.sync.dma_start(out=recv1[RT:RT + 1, :], in_=zrow[:])
            nc.sync.dma_start(out=recv2[RT:RT + 1, :], in_=zrow[:])

            maakb = misc.tile([P, D], F32)
            nc.sync.dma_start(out=maakb[:], in_=maa_k[:].to_broadcast([P, D]))
            maarb = misc.tile([P, D], F32)
            nc.sync.dma_start(out=maarb[:], in_=maa_r[:].to_broadcast([P, D]))

            s32 = misc.tile([P, T // P], I32)
            nc.sync.dma_start(out=s32[:], in_=src_idx32[:])
            s16 = misc.tile([P, T // 16], I16)
            nc.sync.dma_start(out=s16[:], in_=src_idx16[:])
            sl16 = misc.tile([P, EPC * C // 16], I16)
            nc.sync.dma_start(out=sl16[:], in_=slot_idx16[:])
            sl32 = misc.tile([P, EPC * C // P], I32)
            nc.sync.dma_start(out=sl32[:], in_=slot_idx32[:])
            io16 = misc.tile([P, T // 16], I16)
            nc.sync.dma_start(out=io16[:], in_=iota16[:])

            # ---------------- phase A: token shift, pack xk, stash xr
            with tc.tile_pool(name="pa", bufs=2) as pa:
                for t in range(T // P):
                    xc = pa.tile([P, D], F32, tag="xc")
                    nc.sync.dma_start(out=xc[:], in_=x_ext[1 + t * P:1 + (t + 1) * P, :])
                    xp = pa.tile([P, D], F32, tag="xp")
                    nc.sync.dma_start(out=xp[:], in_=x_ext[t * P:(t + 1) * P, :])
                    dx = pa.tile([P, D], F32, tag="dx")
                    nc.vector.tensor_sub(out=dx[:], in0=xp[:], in1=xc[:])
                    tmp = pa.tile([P, D], F32, tag="tmp")
                    xr = pa.tile([P, D], BF16, tag="xr")
                    nc.vector.tensor_mul(out=tmp[:], in0=dx[:], in1=maarb[:])
                    nc.vector.tensor_add(out=xr[:], in0=tmp[:], in1=xc[:])
                    nc.sync.dma_start(out=xr_buf[t * P:(t + 1) * P, :], in_=xr[:])
                    tmp2 = pa.tile([P, D], F32, tag="tmp2")
                    xk = pa.tile([P, D], BF16, tag="xk")
                    nc.vector.tensor_mul(out=tmp2[:], in0=dx[:], in1=maakb[:])
                    nc.vector.tensor_add(out=xk[:], in0=tmp2[:], in1=xc[:])
                    nc.gpsimd.indirect_dma_start(
                        out=a2a1_in[:],
                        out_offset=bass.IndirectOffsetOnAxis(ap=s32[:, t:t + 1], axis=0),
                        in_=xk[:], in_offset=None)

            # ---------------- A2A #1 (dispatch)
            nc.gpsimd.collective_compute(
                "AllToAll", mybir.AluOpType.bypass, replica_groups=rg,
                ins=[a2a1_in[0:RT, :]], outs=[recv1[0:RT, :]])

            # ---------------- phase B: receptance (overlaps A2A #1)
            with (
                tc.tile_pool(name="prw", bufs=1) as prw,
                tc.tile_pool(name="prx", bufs=2) as prx,
            ):
                wrt_sb = prw.tile([P, DC, D], BF16)
                nc.sync.dma_start(out=wrt_sb[:],
                                  in_=wrt.rearrange("(c p) e -> p c e", p=P))
                for ck in range(T // 512):
                    xrT = prx.tile([P, DC, 512], BF16, tag="xrT")
                    nc.gpsimd.dma_gather(
                        out_ap=xrT[:], in_ap=xr_buf[:],
                        idxs_ap=io16[:, ck * 32:(ck + 1) * 32],
                        num_idxs=512, num_idxs_reg=512, elem_size=D,
                        transpose=True)
                    for tt in range(4):
                        for eh in range(2):
                            pr = psr.tile([P, 512], F32, space="PSUM", tag="pr")
                            for dc in range(DC):
                                nc.tensor.matmul(
                                    out=pr[:],
                                    lhsT=xrT[:, dc, tt * P:(tt + 1) * P],
                                    rhs=wrt_sb[:, dc, eh * 512:(eh + 1) * 512],
                                    start=(dc == 0), stop=(dc == DC - 1))
                            rsb = prx.tile([P, 512], BF16, tag="rsb")
                            nc.scalar.activation(out=rsb[:], in_=pr[:],
                                                 func=AF.Sigmoid)
                            r0 = ck * 512 + tt * P
                            nc.sync.dma_start(
                                out=r_buf[r0:r0 + P, eh * 512:(eh + 1) * 512],
                                in_=rsb[:])

            # ---------------- phase C: expert FFNs
            with (
                tc.tile_pool(name="pwk", bufs=2) as pwk,
                tc.tile_pool(name="pwv", bufs=1) as pwv,
                tc.tile_pool(name="pfx", bufs=2) as pfx,
                tc.tile_pool(name="pfh", bufs=2) as pfh,
                tc.tile_pool(name="pfy", bufs=2) as pfy,
            ):
                for el in range(EPC):
                    wk_sb = pwk.tile([P, DC, F], BF16, tag="wk")
                    nc.sync.dma_start(out=wk_sb[:],
                                      in_=wk[el].rearrange("(c p) f -> p c f", p=P))
                    wv_sb = pwv.tile([P, FC, D], BF16, tag="wv")
                    nc.sync.dma_start(out=wv_sb[:],
                                      in_=wv[el].rearrange("(c p) f -> p c f", p=P))
                    for ck in range(C // 512):
                        XT = pfx.tile([P, DC, 512], BF16, tag="XT")
                        col0 = (el * C + ck * 512) // 16
                        nc.gpsimd.dma_gather(
                            out_ap=XT[:], in_ap=recv1[:],
                            idxs_ap=sl16[:, col0:col0 + 32],
                            num_idxs=512, num_idxs_reg=512, elem_size=D,
                            transpose=True)
                        ht = pfh.tile([P, FC, 512], BF16, tag="ht")
                        for ft in range(FC):
                            ph = psh.tile([P, 512], F32, space="PSUM", tag="ph")
                            for dc in range(DC):
                                nc.tensor.matmul(
                                    out=ph[:],
                                    lhsT=wk_sb[:, dc, ft * P:(ft + 1) * P],
                                    rhs=XT[:, dc, :],
                                    start=(dc == 0), stop=(dc == DC - 1))
                            hr = pfh.tile([P, 512], BF16, tag="hr")
                            nc.scalar.activation(out=hr[:], in_=ph[:], func=AF.Relu)
                            nc.vector.tensor_mul(out=ht[:, ft, :], in0=hr[:], in1=hr[:])
                        for tt in range(4):
                            ysb = pfy.tile([P, D], BF16, tag="ysb")
                            for nck in range(2):
                                py = psy.tile([P, 512], F32, space="PSUM", tag="py")
                                for fc in range(FC):
                                    nc.tensor.matmul(
                                        out=py[:],
                                        lhsT=ht[:, fc, tt * P:(tt + 1) * P],
                                        rhs=wv_sb[:, fc, nck * 512:(nck + 1) * 512],
                                        start=(fc == 0), stop=(fc == FC - 1))
                                nc.vector.tensor_copy(
                                    out=ysb[:, nck * 512:(nck + 1) * 512], in_=py[:])
                            scol = el * (C // P) + ck * 4 + tt
                            nc.gpsimd.indirect_dma_start(
                                out=a2a2_in[:],
                                out_offset=bass.IndirectOffsetOnAxis(
                                    ap=sl32[:, scol:scol + 1], axis=0),
                                in_=ysb[:], in_offset=None)

            # ---------------- A2A #2 (combine)
            nc.gpsimd.collective_compute(
                "AllToAll", mybir.AluOpType.bypass, replica_groups=rg,
                ins=[a2a2_in[0:RT, :]], outs=[recv2[0:RT, :]])

            # ---------------- phase D: gather own rows, multiply by r
            with tc.tile_pool(name="pd", bufs=2) as pd:
                for ck in range(T // 512):
                    yg = pd.tile([P, 4, D], BF16, tag="yg")
                    nc.gpsimd.dma_gather(
                        out_ap=yg[:], in_ap=recv2[:],
                        idxs_ap=s16[:, ck * 32:(ck + 1) * 32],
                        num_idxs=512, num_idxs_reg=512, elem_size=D,
                        transpose=False)
                    for a in range(4):
                        rt_ = pd.tile([P, D], BF16, tag="rt")
                        r0 = ck * 512 + a * P
                        nc.sync.dma_start(out=rt_[:], in_=r_buf[r0:r0 + P, :])
                        yo = pd.tile([P, D], F32, tag="yo")
                        nc.vector.tensor_mul(out=yo[:], in0=yg[:, a, :], in1=rt_[:])
                        nc.sync.dma_start(out=out[r0:r0 + P, :], in_=yo[:])

    nc.finalize()
    return nc


# ----------------------------------------------------------------- entry point
def _prepare_inputs(x, token_ids, shift_state, time_maa_k, time_maa_r,
                    w_recept, w_key, w_value):
    K, idxs = _build_indices(token_ids)
    x = np.asarray(x, np.float32)
    shift = np.asarray(shift_state, np.float32)
    wrt = np.ascontiguousarray(np.asarray(w_recept, np.float32).T).astype(nbf16)
    wkb = np.asarray(w_key, np.float32).astype(nbf16)
    wvb = np.asarray(w_value, np.float32).astype(nbf16)
    mk = np.asarray(time_maa_k, np.float32)[None, :]
    mr = np.asarray(time_maa_r, np.float32)[None, :]
    iota = np.tile(np.arange(T, dtype=np.int16).reshape(-1, 16).T, (8, 1))

    in_maps = []
    for k in range(NCORES):
        x_ext = np.concatenate([shift[k:k + 1], x[k]], axis=0)
        in_maps.append({
            "x_ext": np.ascontiguousarray(x_ext),
            "maa_k": mk, "maa_r": mr, "wrt": wrt,
            "wk": np.ascontiguousarray(wkb[EPC * k:EPC * (k + 1)]),
            "wv": np.ascontiguousarray(wvb[EPC * k:EPC * (k + 1)]),
            "iota16": iota,
            **idxs[k],
        })
    return K, in_maps


def kernel(x, token_ids, shift_state, time_maa_k, time_maa_r,
           w_recept, w_key, w_value, _trace=False):
    K, in_maps = _prepare_inputs(x, token_ids, shift_state, time_maa_k,
                                 time_maa_r, w_recept, w_key, w_value)
    if K not in _CACHE:
        _CACHE[K] = _build_nc(K)
    nc = _CACHE[K]
    res = run_bass_kernel_spmd(nc, in_maps, core_ids=list(range(NCORES)),
                               trace=_trace)
    kernel.last_result = res
    y = np.stack([res.results[k]["out"] for k in range(NCORES)], axis=0)
    return y.astype(np.float32)
